# revision 8
# baseline (speedup 1.0000x reference)
"""BiMamba block Trainium2 kernel.

Sharding: 8 cores = 4 batch x 2 directions (fwd/bwd). Each core runs the
full Mamba pass for one (batch, direction) pair in d-major layout
[d_inner partitions, seq free]; the selective scan uses the hardware
tensor_tensor_scan instruction with the state dims processed as separate
scans, exp(delta*A[d,n]) on ScalarE with per-partition scale, and the
n-reduction via accumulating identity matmuls on TensorE. No collectives;
the residual add + fwd/bwd sum happen on host in fp32.
"""

import numpy as np
import ml_dtypes
from contextlib import ExitStack

import concourse.bass as bass
import concourse.mybir as mybir
import concourse.tile as tile
import concourse.bacc as bacc
from concourse.bass_utils import run_bass_kernel_spmd

F32 = mybir.dt.float32
BF16 = mybir.dt.bfloat16
AF = mybir.ActivationFunctionType
OP = mybir.AluOpType
NP_BF16 = ml_dtypes.bfloat16

L_FULL, DM_FULL, DI_FULL, N_FULL, R_FULL, W_FULL = 2048, 1024, 2048, 16, 64, 4


def build_program(L=L_FULL, DM=DM_FULL, DI=DI_FULL, N=N_FULL, R=R_FULL, W=W_FULL,
                  NCHUNK=2, n_cores=8, sim_safe=False):
    P = 128
    DM_T, DI_T = DM // P, DI // P
    Lc = L // NCHUNK
    CS = min(512, L)       # matmul free-dim chunk (one PSUM bank)
    CSc = min(512, Lc)
    # x_proj output row layout, 32-aligned so PSUM partition slices are legal
    off_B = (R + 31) // 32 * 32
    off_C = (off_B + N + 31) // 32 * 32
    M_XP = off_C + N
    assert DM % P == 0 and DI % P == 0 and L % NCHUNK == 0 and Lc % CSc == 0

    nc = bacc.Bacc("TRN2", target_bir_lowering=False, debug=False,
                   num_devices=n_cores)

    xT = nc.dram_tensor("xT", [DM, L], BF16, kind="ExternalInput").ap()
    w_inproj = nc.dram_tensor("w_inproj", [DM, 2 * DI], BF16, kind="ExternalInput").ap()
    w_xproj = nc.dram_tensor("w_xproj", [DI, M_XP], BF16, kind="ExternalInput").ap()
    w_dtproj = nc.dram_tensor("w_dtproj", [R, DI], BF16, kind="ExternalInput").ap()
    w_outproj = nc.dram_tensor("w_outproj", [DI, DM], BF16, kind="ExternalInput").ap()
    cw_in = nc.dram_tensor("cw_sb", [P, DI_T * W], F32, kind="ExternalInput").ap()
    cb_in = nc.dram_tensor("cb_sb", [P, DI_T], F32, kind="ExternalInput").ap()
    dtb_in = nc.dram_tensor("dtb_sb", [P, DI_T], F32, kind="ExternalInput").ap()
    dd_in = nc.dram_tensor("dd_sb", [P, DI_T], F32, kind="ExternalInput").ap()
    a_in = nc.dram_tensor("a_sb", [P, DI_T * N], F32, kind="ExternalInput").ap()
    g_in = nc.dram_tensor("g_sb", [P, DM_T], F32, kind="ExternalInput").ap()
    b_in = nc.dram_tensor("b_sb", [P, DM_T], F32, kind="ExternalInput").ap()
    ident_in = nc.dram_tensor("ident", [P, P], BF16, kind="ExternalInput").ap()

    outT = nc.dram_tensor("outT", [DM, L], F32, kind="ExternalOutput").ap()

    xc_dram = nc.dram_tensor("xc_dram", [DI, L], BF16)
    sz_dram = nc.dram_tensor("sz_dram", [DI, L], BF16)
    bc_dram = nc.dram_tensor("bc_dram", [N, L], BF16)
    cc_dram = nc.dram_tensor("cc_dram", [N, L], BF16)

    with tile.TileContext(nc) as tc, ExitStack() as octx:
        consts = octx.enter_context(tc.tile_pool(name="consts", bufs=1))

        # ---- constants in SBUF ----
        cw_sb = consts.tile([P, DI_T * W], F32); nc.sync.dma_start(cw_sb[:], cw_in[:])
        cb_sb = consts.tile([P, DI_T], F32); nc.sync.dma_start(cb_sb[:], cb_in[:])
        dtb_sb = consts.tile([P, DI_T], F32); nc.sync.dma_start(dtb_sb[:], dtb_in[:])
        dd_sb = consts.tile([P, DI_T], F32); nc.sync.dma_start(dd_sb[:], dd_in[:])
        a_sb = consts.tile([P, DI_T * N], F32); nc.sync.dma_start(a_sb[:], a_in[:])
        g_sb = consts.tile([P, DM_T], F32); nc.sync.dma_start(g_sb[:], g_in[:])
        b_sb = consts.tile([P, DM_T], F32); nc.sync.dma_start(b_sb[:], b_in[:])
        ident = consts.tile([P, P], BF16); nc.sync.dma_start(ident[:], ident_in[:])
        ones_col = consts.tile([P, 1], BF16); nc.vector.memset(ones_col[:], 1.0)
        ones_row = consts.tile([1, P], F32); nc.vector.memset(ones_row[:], 1.0)
        dt_sb = consts.tile([R, L], BF16, tag="dt_sb")
        carry = consts.tile([P, DI_T * N], F32, tag="carry")
        nc.vector.memset(carry[:], 0.0)

        es_xn = ExitStack()
        xnpool = es_xn.enter_context(tc.tile_pool(name="xn", bufs=DM_T))

        # ---- Phase 0: LayerNorm (d-major; mean/var via ones-matmul over partitions)
        xn = []
        with tc.tile_pool(name="p0", bufs=2) as p0, \
             tc.tile_pool(name="p0s", bufs=2) as p0s, \
             tc.tile_pool(name="p0b", bufs=1) as p0b, \
             tc.tile_pool(name="p0x", bufs=DM_T) as p0x, \
             tc.tile_pool(name="ps_ln", bufs=2, space="PSUM") as ps_ln:
            x_sb = []
            for k in range(DM_T):
                t = p0x.tile([P, L], BF16)
                nc.sync.dma_start(t[:], xT[k * P:(k + 1) * P, :])
                x_sb.append(t)
            eps_t = p0b.tile([1, 1], F32, tag="eps")
            nc.vector.memset(eps_t[:], 1e-5)
            # per-column-chunk stats, immediately broadcast to all partitions
            rstd_b = p0b.tile([P, L], F32, tag="rstd_b")
            m2_b = p0b.tile([P, L], F32, tag="m2_b")
            for c in range(L // CS):
                sl = slice(c * CS, (c + 1) * CS)
                ps1 = ps_ln.tile([1, CS], F32, tag="ps_s")
                ps2 = ps_ln.tile([1, CS], F32, tag="ps_s2")
                for k in range(DM_T):
                    sq = p0.tile([P, CS], BF16, tag="sq")
                    nc.scalar.activation(sq[:], x_sb[k][:, sl], AF.Square)
                    nc.tensor.matmul(ps1[:], ones_col[:], x_sb[k][:, sl],
                                     start=(k == 0), stop=(k == DM_T - 1))
                    nc.tensor.matmul(ps2[:], ones_col[:], sq[:],
                                     start=(k == 0), stop=(k == DM_T - 1))
                mu_c = p0s.tile([1, CS], F32, tag="mu_c")
                nc.scalar.mul(mu_c[:], ps1[:], 1.0 / DM)
                ms_c = p0s.tile([1, CS], F32, tag="ms_c")
                nc.scalar.mul(ms_c[:], ps2[:], 1.0 / DM)
                mu2_c = p0s.tile([1, CS], F32, tag="mu2_c")
                nc.scalar.activation(mu2_c[:], mu_c[:], AF.Square)
                var_c = p0s.tile([1, CS], F32, tag="var_c")
                nc.vector.tensor_tensor(var_c[:], ms_c[:], mu2_c[:], OP.subtract)
                std_c = p0s.tile([1, CS], F32, tag="std_c")
                nc.scalar.activation(std_c[:], var_c[:], AF.Sqrt, bias=eps_t[:])
                rstd_c = p0s.tile([1, CS], F32, tag="rstd_c")
                nc.vector.reciprocal(rstd_c[:], std_c[:])
                m2_c = p0s.tile([1, CS], F32, tag="m2_c")
                nc.vector.tensor_tensor(m2_c[:], mu_c[:], rstd_c[:], OP.mult)
                for (src, dst) in ((rstd_c, rstd_b), (m2_c, m2_b)):
                    pb = ps_ln.tile([P, CS], F32, tag="ps_bc")
                    nc.tensor.matmul(pb[:], ones_row[:], src[:],
                                     start=True, stop=True)
                    nc.scalar.copy(dst[:, sl], pb[:])
            for k in range(DM_T):
                t1 = p0.tile([P, L], BF16, tag="ln1")
                nc.vector.tensor_tensor(t1[:], x_sb[k][:], rstd_b[:], OP.mult)
                t2 = p0.tile([P, L], BF16, tag="ln2")
                nc.vector.tensor_tensor(t2[:], t1[:], m2_b[:], OP.subtract)
                t3 = xnpool.tile([P, L], BF16)
                nc.vector.tensor_scalar(t3[:], t2[:], g_sb[:, k:k + 1],
                                        b_sb[:, k:k + 1], OP.mult, OP.add)
                xn.append(t3)

        # ---- Phase 1: in_proj (+ causal depthwise conv + silu), z silu
        with tc.tile_pool(name="p1", bufs=2) as p1, \
             tc.tile_pool(name="w1", bufs=4) as w1, \
             tc.tile_pool(name="ps_xz", bufs=2, space="PSUM") as ps_xz:
            for m in range(2 * DI_T):
                pxz = ps_xz.tile([P, L], F32)
                for k in range(DM_T):
                    wt = w1.tile([P, P], BF16, tag="w_in")
                    nc.sync.dma_start(wt[:],
                                      w_inproj[k * P:(k + 1) * P, m * P:(m + 1) * P])
                    for c in range(L // CS):
                        sl = slice(c * CS, (c + 1) * CS)
                        nc.tensor.matmul(pxz[:, sl], wt[:], xn[k][:, sl],
                                         start=(k == 0), stop=(k == DM_T - 1))
                if m < DI_T:
                    xi = p1.tile([P, W - 1 + L], BF16, tag="xi")
                    nc.vector.memset(xi[:, 0:W - 1], 0.0)
                    nc.scalar.copy(xi[:, W - 1:], pxz[:])
                    cv = p1.tile([P, L], BF16, tag="cv")
                    nc.vector.tensor_scalar(cv[:], xi[:, 0:L],
                                            cw_sb[:, m * W:m * W + 1], None, OP.mult)
                    for w in range(1, W):
                        cv2 = p1.tile([P, L], BF16, tag="cv")
                        nc.vector.scalar_tensor_tensor(
                            cv2[:], xi[:, w:w + L],
                            cw_sb[:, m * W + w:m * W + w + 1], cv[:], OP.mult, OP.add)
                        cv = cv2
                    xct = p1.tile([P, L], BF16, tag="xct")
                    if sim_safe:
                        sg = p1.tile([P, L], BF16, tag="sg")
                        nc.scalar.activation(sg[:], cv[:], AF.Sigmoid,
                                             bias=cb_sb[:, m:m + 1])
                        cvb = p1.tile([P, L], BF16, tag="cvb")
                        nc.vector.tensor_scalar(cvb[:], cv[:], cb_sb[:, m:m + 1],
                                                None, OP.add)
                        nc.vector.tensor_tensor(xct[:], cvb[:], sg[:], OP.mult)
                    else:
                        nc.scalar.activation(xct[:], cv[:], AF.Silu,
                                             bias=cb_sb[:, m:m + 1])
                    nc.sync.dma_start(xc_dram[m * P:(m + 1) * P, :], xct[:])
                else:
                    mz = m - DI_T
                    szt = p1.tile([P, L], BF16, tag="sz")
                    if sim_safe:
                        sg2 = p1.tile([P, L], BF16, tag="sg2")
                        nc.scalar.activation(sg2[:], pxz[:], AF.Sigmoid)
                        nc.vector.tensor_tensor(szt[:], pxz[:], sg2[:], OP.mult)
                    else:
                        nc.scalar.activation(szt[:], pxz[:], AF.Silu)
                    nc.sync.dma_start(sz_dram[mz * P:(mz + 1) * P, :], szt[:])
        es_xn.close()

        # ---- Phase 2: x_proj -> dt rows [0,R), B rows [R,R+N), C rows [R+N,R+2N)
        with tc.tile_pool(name="p2", bufs=3) as p2, \
             tc.tile_pool(name="ps_xp", bufs=1, space="PSUM") as ps_xp:
            pxp = ps_xp.tile([M_XP, L], F32)
            for k in range(DI_T):
                wx = p2.tile([P, M_XP], BF16, tag="w_xp")
                nc.sync.dma_start(wx[:], w_xproj[k * P:(k + 1) * P, :])
                xck = p2.tile([P, L], BF16, tag="xck")
                nc.sync.dma_start(xck[:], xc_dram[k * P:(k + 1) * P, :])
                for c in range(L // CS):
                    sl = slice(c * CS, (c + 1) * CS)
                    nc.tensor.matmul(pxp[:, sl], wx[:], xck[:, sl],
                                     start=(k == 0), stop=(k == DI_T - 1))
            nc.scalar.copy(dt_sb[:], pxp[0:R, :])
            bc_row = p2.tile([N, L], BF16, tag="bc_row")
            nc.scalar.copy(bc_row[:], pxp[off_B:off_B + N, :])
            nc.sync.dma_start(bc_dram[:, :], bc_row[:])
            cc_row = p2.tile([N, L], BF16, tag="cc_row")
            nc.scalar.copy(cc_row[:], pxp[off_C:off_C + N, :])
            nc.sync.dma_start(cc_dram[:, :], cc_row[:])

        # ---- Phase 3: scan + gate + out_proj, chunked along L
        with tc.tile_pool(name="p3", bufs=2) as p3, \
             tc.tile_pool(name="scanp", bufs=3) as scanp, \
             tc.tile_pool(name="w3", bufs=4) as w3, \
             tc.tile_pool(name="ygp", bufs=DI_T) as ygp, \
             tc.tile_pool(name="bcast", bufs=1) as bcast, \
             tc.tile_pool(name="ps_dt", bufs=1, space="PSUM") as ps_dt, \
             tc.tile_pool(name="ps_y", bufs=1, space="PSUM") as ps_y, \
             tc.tile_pool(name="ps_o", bufs=2, space="PSUM") as ps_o:
            for ch in range(NCHUNK):
                lsl = slice(ch * Lc, (ch + 1) * Lc)
                Bb = bcast.tile([P, N * Lc], BF16, tag="Bb")
                nc.sync.dma_start(
                    Bb[:], bc_dram[:, lsl].unsqueeze(0).broadcast_to([P, N, Lc]))
                Cb = bcast.tile([P, N * Lc], BF16, tag="Cb")
                nc.sync.dma_start(
                    Cb[:], cc_dram[:, lsl].unsqueeze(0).broadcast_to([P, N, Lc]))
                yg_sb = []
                for t in range(DI_T):
                    wdt = w3.tile([R, P], BF16, tag="w_dt")
                    nc.sync.dma_start(wdt[:], w_dtproj[:, t * P:(t + 1) * P])
                    pdt = ps_dt.tile([P, Lc], F32)
                    for c in range(Lc // CSc):
                        sl = slice(c * CSc, (c + 1) * CSc)
                        gsl = slice(ch * Lc + c * CSc, ch * Lc + (c + 1) * CSc)
                        nc.tensor.matmul(pdt[:, sl], wdt[:], dt_sb[:, gsl],
                                         start=True, stop=True)
                    # softplus(x + b) = ln(1 + exp(x + b))
                    edt = p3.tile([P, Lc], F32, tag="edt")
                    nc.scalar.activation(edt[:], pdt[:], AF.Exp,
                                         bias=dtb_sb[:, t:t + 1])
                    delta = p3.tile([P, Lc], F32, tag="delta")
                    nc.scalar.activation(delta[:], edt[:], AF.Ln, bias=1.0)
                    xcch = p3.tile([P, Lc], BF16, tag="xcch")
                    nc.sync.dma_start(xcch[:], xc_dram[t * P:(t + 1) * P, lsl])
                    szch = p3.tile([P, Lc], BF16, tag="szch")
                    nc.sync.dma_start(szch[:], sz_dram[t * P:(t + 1) * P, lsl])
                    du = p3.tile([P, Lc], BF16, tag="du")
                    nc.vector.tensor_tensor(du[:], delta[:], xcch[:], OP.mult)
                    py = ps_y.tile([P, Lc], F32)
                    for n in range(N):
                        nsl = slice(n * Lc, (n + 1) * Lc)
                        cidx = t * N + n
                        dA = scanp.tile([P, Lc], BF16, tag="dA")
                        nc.scalar.activation(dA[:], delta[:], AF.Exp,
                                             scale=a_sb[:, cidx:cidx + 1])
                        dBu = scanp.tile([P, Lc], BF16, tag="dBu")
                        nc.vector.tensor_tensor(dBu[:], du[:], Bb[:, nsl], OP.mult)
                        h = scanp.tile([P, Lc], BF16, tag="h")
                        nc.vector.tensor_tensor_scan(
                            h[:], dA[:], dBu[:], carry[:, cidx:cidx + 1],
                            OP.mult, OP.add)
                        if ch < NCHUNK - 1:
                            nc.scalar.copy(carry[:, cidx:cidx + 1], h[:, Lc - 1:Lc])
                        hC = scanp.tile([P, Lc], BF16, tag="hC")
                        nc.vector.tensor_tensor(hC[:], h[:], Cb[:, nsl], OP.mult)
                        for c in range(Lc // CSc):
                            sl = slice(c * CSc, (c + 1) * CSc)
                            nc.tensor.matmul(py[:, sl], ident[:], hC[:, sl],
                                             start=(n == 0), stop=(n == N - 1))
                    t1 = p3.tile([P, Lc], BF16, tag="gate1")
                    nc.vector.scalar_tensor_tensor(t1[:], xcch[:], dd_sb[:, t:t + 1],
                                                   py[:], OP.mult, OP.add)
                    ygt = ygp.tile([P, Lc], BF16)
                    nc.vector.tensor_tensor(ygt[:], t1[:], szch[:], OP.mult)
                    yg_sb.append(ygt)
                for m in range(DM_T):
                    po = ps_o.tile([P, Lc], F32)
                    for k in range(DI_T):
                        wo = w3.tile([P, P], BF16, tag="w_out")
                        nc.sync.dma_start(wo[:], w_outproj[k * P:(k + 1) * P,
                                                           m * P:(m + 1) * P])
                        for c in range(Lc // CSc):
                            sl = slice(c * CSc, (c + 1) * CSc)
                            nc.tensor.matmul(po[:, sl], wo[:], yg_sb[k][:, sl],
                                             start=(k == 0), stop=(k == DI_T - 1))
                    ot = p3.tile([P, Lc], F32, tag="ot")
                    nc.scalar.copy(ot[:], po[:])
                    nc.sync.dma_start(outT[m * P:(m + 1) * P, lsl], ot[:])

    nc.compile()
    return nc


def _pack_cols(v, P=128):
    # [T*P] -> [P, T] (or [T*P, K] -> [P, T*K]) partition-major packing
    v = np.asarray(v, np.float32)
    if v.ndim == 1:
        T = v.shape[0] // P
        return np.ascontiguousarray(v.reshape(T, P).T)
    T = v.shape[0] // P
    K = v.shape[1]
    return np.ascontiguousarray(
        v.reshape(T, P, K).transpose(1, 0, 2).reshape(P, T * K))


def _xproj_padded(x_proj_w, R, N):
    wt = np.asarray(x_proj_w, np.float32).T  # [DI, R+2N]
    DI = wt.shape[0]
    off_B = (R + 31) // 32 * 32
    off_C = (off_B + N + 31) // 32 * 32
    out = np.zeros((DI, off_C + N), np.float32)
    out[:, 0:R] = wt[:, 0:R]
    out[:, off_B:off_B + N] = wt[:, R:R + N]
    out[:, off_C:off_C + N] = wt[:, R + N:R + 2 * N]
    return out


def _core_inputs(params):
    p = {k: np.asarray(v) for k, v in params.items()}
    R = p["dt_proj_w"].shape[1]
    N = p["A_log"].shape[1]
    return {
        "w_inproj": np.ascontiguousarray(p["in_proj_w"].T).astype(NP_BF16),
        "w_xproj": np.ascontiguousarray(_xproj_padded(p["x_proj_w"], R, N)).astype(NP_BF16),
        "w_dtproj": np.ascontiguousarray(p["dt_proj_w"].T).astype(NP_BF16),
        "w_outproj": np.ascontiguousarray(p["out_proj_w"].T).astype(NP_BF16),
        "cw_sb": _pack_cols(p["conv_w"]),
        "cb_sb": _pack_cols(p["conv_b"]),
        "dtb_sb": _pack_cols(p["dt_proj_b"]),
        "dd_sb": _pack_cols(p["D"]),
        "a_sb": _pack_cols(-np.exp(np.asarray(p["A_log"], np.float32))),
        "ident": np.eye(128, dtype=NP_BF16),
    }


_PROGRAM_CACHE = {}


def kernel(x, norm_g, norm_b, params_fwd, params_bwd):
    x = np.asarray(x, np.float32)
    B, L, DM = x.shape
    pf = {k: np.asarray(v) for k, v in params_fwd.items()}
    pb = {k: np.asarray(v) for k, v in params_bwd.items()}
    DI = pf["in_proj_w"].shape[0] // 2
    R = pf["dt_proj_w"].shape[1]
    N = pf["A_log"].shape[1]
    W = pf["conv_w"].shape[1]
    n_cores = 2 * B
    key = (L, DM, DI, N, R, W, n_cores)
    if key not in _PROGRAM_CACHE:
        _PROGRAM_CACHE[key] = build_program(L, DM, DI, N, R, W,
                                            NCHUNK=2, n_cores=n_cores)
    nc = _PROGRAM_CACHE[key]

    g_sb = _pack_cols(np.asarray(norm_g, np.float32))
    b_sb = _pack_cols(np.asarray(norm_b, np.float32))
    fwd_common = _core_inputs(pf)
    bwd_common = _core_inputs(pb)
    in_maps = []
    for b in range(B):
        m = dict(fwd_common)
        m["xT"] = np.ascontiguousarray(x[b].T).astype(NP_BF16)
        m["g_sb"] = g_sb
        m["b_sb"] = b_sb
        in_maps.append(m)
    for b in range(B):
        m = dict(bwd_common)
        m["xT"] = np.ascontiguousarray(x[b][::-1, :].T).astype(NP_BF16)
        m["g_sb"] = g_sb
        m["b_sb"] = b_sb
        in_maps.append(m)

    res = run_bass_kernel_spmd(nc, in_maps, core_ids=list(range(n_cores)))
    out = x.copy()
    for b in range(B):
        out[b] += res.results[b]["outT"].T
        out[b] += res.results[B + b]["outT"].T[::-1, :]
    return out


# revision 11
# speedup vs baseline: 1.0100x; 1.0100x over previous
"""BiMamba block Trainium2 kernel.

Sharding: 8 cores = 4 batch x 2 directions (fwd/bwd). Each core runs the
full Mamba pass for one (batch, direction) pair in d-major layout
[d_inner partitions, seq free]; the selective scan uses the hardware
tensor_tensor_scan instruction with the state dims processed as separate
scans, exp(delta*A[d,n]) on ScalarE with per-partition scale, and the
n-reduction via accumulating identity matmuls on TensorE. No collectives;
the residual add + fwd/bwd sum happen on host in fp32.
"""

import numpy as np
import ml_dtypes
from contextlib import ExitStack

import concourse.bass as bass
import concourse.mybir as mybir
import concourse.tile as tile
import concourse.bacc as bacc
import concourse.hw_specs as _hw_specs
from concourse.bass_utils import run_bass_kernel_spmd

# Route exp/ln (and the every-set fillers copy/square/identity) to the single
# natural_log_exp_and_others table set, and silu to silu_and_others, so the
# scheduler does not ping-pong ACT table loads between sets that each hold
# only half of a phase's functions.
_ORIG_ACT_TABLES = _hw_specs.get_activation_tables


def _patched_act_tables(arch):
    t = _ORIG_ACT_TABLES(arch)
    out = {}
    for name, fns in t.items():
        if name in ("exp_and_others", "softplus_and_others", "sigmoid_and_others",
                    "small", "natural_log"):
            out[name] = set()
        elif name == "sqrt_and_others":
            out[name] = {f for f in fns
                         if f == mybir.ActivationFunctionType.Sqrt}
        else:
            out[name] = fns
    return out


bacc.get_activation_tables = _patched_act_tables

F32 = mybir.dt.float32
BF16 = mybir.dt.bfloat16
AF = mybir.ActivationFunctionType
OP = mybir.AluOpType
NP_BF16 = ml_dtypes.bfloat16

L_FULL, DM_FULL, DI_FULL, N_FULL, R_FULL, W_FULL = 2048, 1024, 2048, 16, 64, 4


def build_program(L=L_FULL, DM=DM_FULL, DI=DI_FULL, N=N_FULL, R=R_FULL, W=W_FULL,
                  NCHUNK=2, n_cores=8, sim_safe=False):
    P = 128
    DM_T, DI_T = DM // P, DI // P
    Lc = L // NCHUNK
    CS = min(512, L)       # matmul free-dim chunk (one PSUM bank)
    CSc = min(512, Lc)
    # x_proj output row layout, 32-aligned so PSUM partition slices are legal
    off_B = (R + 31) // 32 * 32
    off_C = (off_B + N + 31) // 32 * 32
    M_XP = off_C + N
    assert DM % P == 0 and DI % P == 0 and L % NCHUNK == 0 and Lc % CSc == 0

    nc = bacc.Bacc("TRN2", target_bir_lowering=False, debug=False,
                   num_devices=n_cores)

    xT = nc.dram_tensor("xT", [DM, L], BF16, kind="ExternalInput").ap()
    w_inproj = nc.dram_tensor("w_inproj", [DM, 2 * DI], BF16, kind="ExternalInput").ap()
    w_xproj = nc.dram_tensor("w_xproj", [DI, M_XP], BF16, kind="ExternalInput").ap()
    w_dtproj = nc.dram_tensor("w_dtproj", [R, DI], BF16, kind="ExternalInput").ap()
    w_outproj = nc.dram_tensor("w_outproj", [DI, DM], BF16, kind="ExternalInput").ap()
    cw_in = nc.dram_tensor("cw_sb", [P, DI_T * W], F32, kind="ExternalInput").ap()
    cb_in = nc.dram_tensor("cb_sb", [P, DI_T], F32, kind="ExternalInput").ap()
    dtb_in = nc.dram_tensor("dtb_sb", [P, DI_T], F32, kind="ExternalInput").ap()
    dd_in = nc.dram_tensor("dd_sb", [P, DI_T], F32, kind="ExternalInput").ap()
    a_in = nc.dram_tensor("a_sb", [P, DI_T * N], F32, kind="ExternalInput").ap()
    g_in = nc.dram_tensor("g_sb", [P, DM_T], F32, kind="ExternalInput").ap()
    b_in = nc.dram_tensor("b_sb", [P, DM_T], F32, kind="ExternalInput").ap()
    ident_in = nc.dram_tensor("ident", [P, P], BF16, kind="ExternalInput").ap()

    outT = nc.dram_tensor("outT", [DM, L], F32, kind="ExternalOutput").ap()

    xc_dram = nc.dram_tensor("xc_dram", [DI, L], BF16)
    sz_dram = nc.dram_tensor("sz_dram", [DI, L], BF16)
    bc_dram = nc.dram_tensor("bc_dram", [N, L], BF16)
    cc_dram = nc.dram_tensor("cc_dram", [N, L], BF16)

    with tile.TileContext(nc) as tc, ExitStack() as octx:
        consts = octx.enter_context(tc.tile_pool(name="consts", bufs=1))

        # ---- constants in SBUF ----
        cw_sb = consts.tile([P, DI_T * W], F32); nc.sync.dma_start(cw_sb[:], cw_in[:])
        cb_sb = consts.tile([P, DI_T], F32); nc.sync.dma_start(cb_sb[:], cb_in[:])
        dtb_sb = consts.tile([P, DI_T], F32); nc.sync.dma_start(dtb_sb[:], dtb_in[:])
        dd_sb = consts.tile([P, DI_T], F32); nc.sync.dma_start(dd_sb[:], dd_in[:])
        a_sb = consts.tile([P, DI_T * N], F32); nc.sync.dma_start(a_sb[:], a_in[:])
        g_sb = consts.tile([P, DM_T], F32); nc.sync.dma_start(g_sb[:], g_in[:])
        b_sb = consts.tile([P, DM_T], F32); nc.sync.dma_start(b_sb[:], b_in[:])
        ident = consts.tile([P, P], BF16); nc.sync.dma_start(ident[:], ident_in[:])
        ones_col = consts.tile([P, 1], BF16); nc.vector.memset(ones_col[:], 1.0)
        ones_row = consts.tile([1, P], F32); nc.vector.memset(ones_row[:], 1.0)
        dt_sb = consts.tile([R, L], BF16, tag="dt_sb")
        carry = consts.tile([P, DI_T * N], F32, tag="carry")
        nc.vector.memset(carry[:], 0.0)

        es_xn = ExitStack()
        xnpool = es_xn.enter_context(tc.tile_pool(name="xn", bufs=DM_T))

        # ---- Phase 0: LayerNorm (d-major; mean/var via ones-matmul over partitions)
        xn = []
        with tc.tile_pool(name="p0", bufs=2) as p0, \
             tc.tile_pool(name="p0s", bufs=2) as p0s, \
             tc.tile_pool(name="p0b", bufs=1) as p0b, \
             tc.tile_pool(name="p0x", bufs=DM_T) as p0x, \
             tc.tile_pool(name="ps_ln", bufs=2, space="PSUM") as ps_ln:
            x_sb = []
            for k in range(DM_T):
                t = p0x.tile([P, L], BF16)
                nc.sync.dma_start(t[:], xT[k * P:(k + 1) * P, :])
                x_sb.append(t)
            eps_t = p0b.tile([1, 1], F32, tag="eps")
            nc.vector.memset(eps_t[:], 1e-5)
            # per-column-chunk stats, immediately broadcast to all partitions
            rstd_b = p0b.tile([P, L], BF16, tag="rstd_b")
            m2_b = p0b.tile([P, L], BF16, tag="m2_b")
            mu_sb = p0b.tile([1, L], F32, tag="mu_sb")
            var_sb = p0b.tile([1, L], F32, tag="var_sb")
            for c in range(L // CS):
                sl = slice(c * CS, (c + 1) * CS)
                ps1 = ps_ln.tile([1, CS], F32, tag="ps_s")
                ps2 = ps_ln.tile([1, CS], F32, tag="ps_s2")
                for k in range(DM_T):
                    sq = p0.tile([P, CS], BF16, tag="sq")
                    nc.scalar.activation(sq[:], x_sb[k][:, sl], AF.Square)
                    nc.tensor.matmul(ps1[:], ones_col[:], x_sb[k][:, sl],
                                     start=(k == 0), stop=(k == DM_T - 1))
                    nc.tensor.matmul(ps2[:], ones_col[:], sq[:],
                                     start=(k == 0), stop=(k == DM_T - 1))
                nc.scalar.mul(mu_sb[:, sl], ps1[:], 1.0 / DM)
                ms_c = p0s.tile([1, CS], F32, tag="ms_c")
                nc.scalar.mul(ms_c[:], ps2[:], 1.0 / DM)
                mu2_c = p0s.tile([1, CS], F32, tag="mu2_c")
                nc.scalar.activation(mu2_c[:], mu_sb[:, sl], AF.Square)
                nc.vector.tensor_tensor(var_sb[:, sl], ms_c[:], mu2_c[:],
                                        OP.subtract)
            std_sb = p0b.tile([1, L], F32, tag="std_sb")
            nc.scalar.activation(std_sb[:], var_sb[:], AF.Sqrt, bias=eps_t[:])
            rstd_sb = p0b.tile([1, L], F32, tag="rstd_sb")
            nc.vector.reciprocal(rstd_sb[:], std_sb[:])
            m2_sb = p0b.tile([1, L], F32, tag="m2_sb")
            nc.vector.tensor_tensor(m2_sb[:], mu_sb[:], rstd_sb[:], OP.mult)
            for (src, dst) in ((rstd_sb, rstd_b), (m2_sb, m2_b)):
                for c in range(L // CS):
                    sl = slice(c * CS, (c + 1) * CS)
                    pb = ps_ln.tile([P, CS], F32, tag="ps_bc")
                    nc.tensor.matmul(pb[:], ones_row[:], src[:, sl],
                                     start=True, stop=True)
                    nc.scalar.copy(dst[:, sl], pb[:])
            for k in range(DM_T):
                t1 = p0.tile([P, L], BF16, tag="ln1")
                nc.vector.tensor_tensor(t1[:], x_sb[k][:], rstd_b[:], OP.mult)
                t2 = p0.tile([P, L], BF16, tag="ln2")
                nc.vector.tensor_tensor(t2[:], t1[:], m2_b[:], OP.subtract)
                t3 = xnpool.tile([P, L], BF16)
                nc.vector.tensor_scalar(t3[:], t2[:], g_sb[:, k:k + 1],
                                        b_sb[:, k:k + 1], OP.mult, OP.add)
                xn.append(t3)

        # ---- Phase 1: in_proj (+ causal depthwise conv + silu), z silu.
        # z-tiles first so their silus batch; xc silus deferred to a tail
        # batch so ACT runs [silu...][copy...][silu...] with 2 table loads.
        with tc.tile_pool(name="p1", bufs=2) as p1, \
             tc.tile_pool(name="w1", bufs=4) as w1, \
             tc.tile_pool(name="cvp", bufs=DI_T) as cvp, \
             tc.tile_pool(name="ps_xz", bufs=2, space="PSUM") as ps_xz:
            cv_done = []
            last_copy = None
            for m in list(range(DI_T, 2 * DI_T)) + list(range(DI_T)):
                pxz = ps_xz.tile([P, L], F32)
                for k in range(DM_T):
                    wt = w1.tile([P, P], BF16, tag="w_in")
                    nc.sync.dma_start(wt[:],
                                      w_inproj[k * P:(k + 1) * P, m * P:(m + 1) * P])
                    for c in range(L // CS):
                        sl = slice(c * CS, (c + 1) * CS)
                        nc.tensor.matmul(pxz[:, sl], wt[:], xn[k][:, sl],
                                         start=(k == 0), stop=(k == DM_T - 1))
                if m < DI_T:
                    xi = p1.tile([P, W - 1 + L], BF16, tag="xi")
                    nc.vector.memset(xi[:, 0:W - 1], 0.0)
                    nc.scalar.copy(xi[:, W - 1:], pxz[:])
                    xi1 = p1.tile([P, W - 2 + L], BF16, tag="xi1")
                    last_copy = nc.scalar.copy(xi1[:], xi[:, 1:])
                    cv = p1.tile([P, L], BF16, tag="cv")
                    nc.vector.tensor_scalar(cv[:], xi[:, 0:L],
                                            cw_sb[:, m * W:m * W + 1], None, OP.mult)
                    for w in range(1, W):
                        src = xi1[:, w - 1:w - 1 + L] if w % 2 else xi[:, w:w + L]
                        if w == W - 1:
                            cv2 = cvp.tile([P, L], BF16, tag="cvf")
                        else:
                            cv2 = p1.tile([P, L], BF16, tag="cv")
                        nc.vector.scalar_tensor_tensor(
                            cv2[:], src,
                            cw_sb[:, m * W + w:m * W + w + 1], cv[:], OP.mult, OP.add)
                        cv = cv2
                    cv_done.append((m, cv))
                else:
                    mz = m - DI_T
                    szt = p1.tile([P, L], BF16, tag="sz")
                    if sim_safe:
                        sg2 = p1.tile([P, L], BF16, tag="sg2")
                        nc.scalar.activation(sg2[:], pxz[:], AF.Sigmoid)
                        nc.vector.tensor_tensor(szt[:], pxz[:], sg2[:], OP.mult)
                    else:
                        nc.scalar.activation(szt[:], pxz[:], AF.Silu)
                    nc.sync.dma_start(sz_dram[mz * P:(mz + 1) * P, :], szt[:])
            for (m, cv) in cv_done:
                xct = p1.tile([P, L], BF16, tag="xct")
                if sim_safe:
                    sg = p1.tile([P, L], BF16, tag="sg")
                    nc.scalar.activation(sg[:], cv[:], AF.Sigmoid,
                                         bias=cb_sb[:, m:m + 1])
                    cvb = p1.tile([P, L], BF16, tag="cvb")
                    nc.vector.tensor_scalar(cvb[:], cv[:], cb_sb[:, m:m + 1],
                                            None, OP.add)
                    nc.vector.tensor_tensor(xct[:], cvb[:], sg[:], OP.mult)
                else:
                    si = nc.scalar.activation(xct[:], cv[:], AF.Silu,
                                              bias=cb_sb[:, m:m + 1])
                    if last_copy is not None:
                        bass._add_dep_helper(si.ins, last_copy.ins, sync=False,
                                             reason="batch silus after copies")
                nc.sync.dma_start(xc_dram[m * P:(m + 1) * P, :], xct[:])
        es_xn.close()

        # ---- Phase 2: x_proj -> dt rows [0,R), B rows [R,R+N), C rows [R+N,R+2N)
        with tc.tile_pool(name="p2", bufs=3) as p2, \
             tc.tile_pool(name="ps_xp", bufs=1, space="PSUM") as ps_xp:
            pxp = ps_xp.tile([M_XP, L], F32)
            for k in range(DI_T):
                wx = p2.tile([P, M_XP], BF16, tag="w_xp")
                nc.sync.dma_start(wx[:], w_xproj[k * P:(k + 1) * P, :])
                xck = p2.tile([P, L], BF16, tag="xck")
                nc.sync.dma_start(xck[:], xc_dram[k * P:(k + 1) * P, :])
                for c in range(L // CS):
                    sl = slice(c * CS, (c + 1) * CS)
                    nc.tensor.matmul(pxp[:, sl], wx[:], xck[:, sl],
                                     start=(k == 0), stop=(k == DI_T - 1))
            nc.scalar.copy(dt_sb[:], pxp[0:R, :])
            bc_row = p2.tile([N, L], BF16, tag="bc_row")
            nc.scalar.copy(bc_row[:], pxp[off_B:off_B + N, :])
            nc.sync.dma_start(bc_dram[:, :], bc_row[:])
            cc_row = p2.tile([N, L], BF16, tag="cc_row")
            nc.scalar.copy(cc_row[:], pxp[off_C:off_C + N, :])
            nc.sync.dma_start(cc_dram[:, :], cc_row[:])

        # ---- Phase 3: scan + gate + out_proj, chunked along L
        with tc.tile_pool(name="p3", bufs=2) as p3, \
             tc.tile_pool(name="scanp", bufs=3) as scanp, \
             tc.tile_pool(name="w3", bufs=4) as w3, \
             tc.tile_pool(name="ygp", bufs=DI_T) as ygp, \
             tc.tile_pool(name="bcast", bufs=1) as bcast, \
             tc.tile_pool(name="ps_dt", bufs=1, space="PSUM") as ps_dt, \
             tc.tile_pool(name="ps_y", bufs=1, space="PSUM") as ps_y, \
             tc.tile_pool(name="ps_o", bufs=2, space="PSUM") as ps_o:
            for ch in range(NCHUNK):
                lsl = slice(ch * Lc, (ch + 1) * Lc)
                Bb = bcast.tile([P, N * Lc], BF16, tag="Bb")
                nc.sync.dma_start(
                    Bb[:], bc_dram[:, lsl].unsqueeze(0).broadcast_to([P, N, Lc]))
                Cb = bcast.tile([P, N * Lc], BF16, tag="Cb")
                nc.sync.dma_start(
                    Cb[:], cc_dram[:, lsl].unsqueeze(0).broadcast_to([P, N, Lc]))
                yg_sb = []
                for t in range(DI_T):
                    wdt = w3.tile([R, P], BF16, tag="w_dt")
                    nc.sync.dma_start(wdt[:], w_dtproj[:, t * P:(t + 1) * P])
                    pdt = ps_dt.tile([P, Lc], F32)
                    for c in range(Lc // CSc):
                        sl = slice(c * CSc, (c + 1) * CSc)
                        gsl = slice(ch * Lc + c * CSc, ch * Lc + (c + 1) * CSc)
                        nc.tensor.matmul(pdt[:, sl], wdt[:], dt_sb[:, gsl],
                                         start=True, stop=True)
                    # softplus(x + b) = ln(1 + exp(x + b))
                    edt = p3.tile([P, Lc], F32, tag="edt")
                    nc.scalar.activation(edt[:], pdt[:], AF.Exp,
                                         bias=dtb_sb[:, t:t + 1])
                    delta = p3.tile([P, Lc], F32, tag="delta")
                    nc.scalar.activation(delta[:], edt[:], AF.Ln, bias=1.0)
                    xcch = p3.tile([P, Lc], BF16, tag="xcch")
                    nc.sync.dma_start(xcch[:], xc_dram[t * P:(t + 1) * P, lsl])
                    szch = p3.tile([P, Lc], BF16, tag="szch")
                    nc.sync.dma_start(szch[:], sz_dram[t * P:(t + 1) * P, lsl])
                    du = p3.tile([P, Lc], BF16, tag="du")
                    nc.vector.tensor_tensor(du[:], delta[:], xcch[:], OP.mult)
                    py = ps_y.tile([P, Lc], F32)
                    for n in range(N):
                        nsl = slice(n * Lc, (n + 1) * Lc)
                        cidx = t * N + n
                        dA = scanp.tile([P, Lc], BF16, tag="dA")
                        nc.scalar.activation(dA[:], delta[:], AF.Exp,
                                             scale=a_sb[:, cidx:cidx + 1])
                        dBu = scanp.tile([P, Lc], BF16, tag="dBu")
                        nc.vector.tensor_tensor(dBu[:], du[:], Bb[:, nsl], OP.mult)
                        h = scanp.tile([P, Lc], BF16, tag="h")
                        nc.vector.tensor_tensor_scan(
                            h[:], dA[:], dBu[:], carry[:, cidx:cidx + 1],
                            OP.mult, OP.add)
                        if ch < NCHUNK - 1:
                            nc.scalar.copy(carry[:, cidx:cidx + 1], h[:, Lc - 1:Lc])
                        hC = scanp.tile([P, Lc], BF16, tag="hC")
                        nc.vector.tensor_tensor(hC[:], h[:], Cb[:, nsl], OP.mult)
                        for c in range(Lc // CSc):
                            sl = slice(c * CSc, (c + 1) * CSc)
                            nc.tensor.matmul(py[:, sl], ident[:], hC[:, sl],
                                             start=(n == 0), stop=(n == N - 1))
                    t1 = p3.tile([P, Lc], BF16, tag="gate1")
                    nc.vector.scalar_tensor_tensor(t1[:], xcch[:], dd_sb[:, t:t + 1],
                                                   py[:], OP.mult, OP.add)
                    ygt = ygp.tile([P, Lc], BF16)
                    nc.vector.tensor_tensor(ygt[:], t1[:], szch[:], OP.mult)
                    yg_sb.append(ygt)
                for m in range(DM_T):
                    po = ps_o.tile([P, Lc], F32)
                    for k in range(DI_T):
                        wo = w3.tile([P, P], BF16, tag="w_out")
                        nc.sync.dma_start(wo[:], w_outproj[k * P:(k + 1) * P,
                                                           m * P:(m + 1) * P])
                        for c in range(Lc // CSc):
                            sl = slice(c * CSc, (c + 1) * CSc)
                            nc.tensor.matmul(po[:, sl], wo[:], yg_sb[k][:, sl],
                                             start=(k == 0), stop=(k == DI_T - 1))
                    ot = p3.tile([P, Lc], F32, tag="ot")
                    nc.scalar.copy(ot[:], po[:])
                    nc.sync.dma_start(outT[m * P:(m + 1) * P, lsl], ot[:])

    nc.compile()
    return nc


def _pack_cols(v, P=128):
    # [T*P] -> [P, T] (or [T*P, K] -> [P, T*K]) partition-major packing
    v = np.asarray(v, np.float32)
    if v.ndim == 1:
        T = v.shape[0] // P
        return np.ascontiguousarray(v.reshape(T, P).T)
    T = v.shape[0] // P
    K = v.shape[1]
    return np.ascontiguousarray(
        v.reshape(T, P, K).transpose(1, 0, 2).reshape(P, T * K))


def _xproj_padded(x_proj_w, R, N):
    wt = np.asarray(x_proj_w, np.float32).T  # [DI, R+2N]
    DI = wt.shape[0]
    off_B = (R + 31) // 32 * 32
    off_C = (off_B + N + 31) // 32 * 32
    out = np.zeros((DI, off_C + N), np.float32)
    out[:, 0:R] = wt[:, 0:R]
    out[:, off_B:off_B + N] = wt[:, R:R + N]
    out[:, off_C:off_C + N] = wt[:, R + N:R + 2 * N]
    return out


def _core_inputs(params):
    p = {k: np.asarray(v) for k, v in params.items()}
    R = p["dt_proj_w"].shape[1]
    N = p["A_log"].shape[1]
    return {
        "w_inproj": np.ascontiguousarray(p["in_proj_w"].T).astype(NP_BF16),
        "w_xproj": np.ascontiguousarray(_xproj_padded(p["x_proj_w"], R, N)).astype(NP_BF16),
        "w_dtproj": np.ascontiguousarray(p["dt_proj_w"].T).astype(NP_BF16),
        "w_outproj": np.ascontiguousarray(p["out_proj_w"].T).astype(NP_BF16),
        "cw_sb": _pack_cols(p["conv_w"]),
        "cb_sb": _pack_cols(p["conv_b"]),
        "dtb_sb": _pack_cols(p["dt_proj_b"]),
        "dd_sb": _pack_cols(p["D"]),
        "a_sb": _pack_cols(-np.exp(np.asarray(p["A_log"], np.float32))),
        "ident": np.eye(128, dtype=NP_BF16),
    }


_PROGRAM_CACHE = {}


def kernel(x, norm_g, norm_b, params_fwd, params_bwd):
    x = np.asarray(x, np.float32)
    B, L, DM = x.shape
    pf = {k: np.asarray(v) for k, v in params_fwd.items()}
    pb = {k: np.asarray(v) for k, v in params_bwd.items()}
    DI = pf["in_proj_w"].shape[0] // 2
    R = pf["dt_proj_w"].shape[1]
    N = pf["A_log"].shape[1]
    W = pf["conv_w"].shape[1]
    n_cores = 2 * B
    key = (L, DM, DI, N, R, W, n_cores)
    if key not in _PROGRAM_CACHE:
        _PROGRAM_CACHE[key] = build_program(L, DM, DI, N, R, W,
                                            NCHUNK=2, n_cores=n_cores)
    nc = _PROGRAM_CACHE[key]

    g_sb = _pack_cols(np.asarray(norm_g, np.float32))
    b_sb = _pack_cols(np.asarray(norm_b, np.float32))
    fwd_common = _core_inputs(pf)
    bwd_common = _core_inputs(pb)
    in_maps = []
    for b in range(B):
        m = dict(fwd_common)
        m["xT"] = np.ascontiguousarray(x[b].T).astype(NP_BF16)
        m["g_sb"] = g_sb
        m["b_sb"] = b_sb
        in_maps.append(m)
    for b in range(B):
        m = dict(bwd_common)
        m["xT"] = np.ascontiguousarray(x[b][::-1, :].T).astype(NP_BF16)
        m["g_sb"] = g_sb
        m["b_sb"] = b_sb
        in_maps.append(m)

    res = run_bass_kernel_spmd(nc, in_maps, core_ids=list(range(n_cores)))
    out = x.copy()
    for b in range(B):
        out[b] += res.results[b]["outT"].T
        out[b] += res.results[B + b]["outT"].T[::-1, :]
    return out
